# revision 18
# baseline (speedup 1.0000x reference)
"""Trainium2 Bass kernel: sparse 7x7x7 stride-1 max-pool over a 64^3 voxel grid
(MinkowskiEngine semantics) + per-point MLP (1x1 conv -> ReLU -> 1x1 conv ->
sigmoid) * feats.

Strategy (8 NeuronCores, SPMD, no collectives):
  - Shard the dense grid along z: core k owns z in [8k, 8k+8), and processes a
    14-plane z-slab (3-voxel halo each side; halo replicated on host -> no
    cross-core exchange).
  - The HOST builds the dense slab in the exact on-chip layout the kernel
    wants: per x-plane [128 part = ch%128, 2 ch-halves, 14 z, 72 y] with y
    pre-padded by 4 sentinel columns each side and empty voxels = -1e30.
    This is pure data marshalling (scatter + transpose + pad); all max/matmul
    arithmetic stays on device.
  - Device streams over the 64 x-planes: contiguous plane load, separable
    windowed max (7 = max of two 4-windows) on DVE along z then y (both in
    aligned 2x-mode layouts), then x across streamed planes; fused MLP on PE
    with the second matmul transposed (W2^T as weights) so sigmoid lands in
    [ch, vox] layout; dense multiply sg * plane-center on DVE; contiguous
    store of the dense product.
  - Host gathers the occupied voxels from the dense product planes (reverse
    marshalling) and casts to fp32.
"""

from contextlib import ExitStack
from dataclasses import dataclass

import numpy as np

C = 256
R = 128
SENT = -1.0e30


@dataclass(frozen=True)
class Cfg:
    D: int = 64           # grid extent per axis
    ZS: int = 8           # owned z-planes per core
    NPTS: int = 100000    # total points
    ncores: int = 8
    dt: str = "bfloat16"  # dtype of planes / pooling / matmul inputs
    PB: int = 2           # planes per build batch (z/y pass batching)

    @property
    def ZH(self):
        return self.ZS + 6

    @property
    def YP(self):
        return self.D + 8  # y padded to 72 (4 sentinel cols each side)

    @property
    def NX(self):
        return self.D

    @property
    def PLF(self):
        return 2 * self.ZH * self.YP  # free elems per partition per plane

    @property
    def VOX2(self):
        return 2 * self.ZS * self.D  # compact pooled free elems (2h * 8z * 64y)


FULL = Cfg()


def _np_dt(name):
    import ml_dtypes

    return {"bfloat16": ml_dtypes.bfloat16, "float16": np.float16}[name]


def build_nc(cfg: Cfg):
    """Build the (SPMD, per-core-identical) Bass program."""
    import concourse.bacc as bacc
    import concourse.tile as tile
    from concourse import mybir

    AF = mybir.ActivationFunctionType
    f32 = mybir.dt.float32
    dt = getattr(mybir.dt, cfg.dt)

    D, ZS, ZH, YP, NX, PB = cfg.D, cfg.ZS, cfg.ZH, cfg.YP, cfg.NX, cfg.PB
    PLF = cfg.PLF
    VOX2 = cfg.VOX2
    VOXH = ZS * D  # voxels per x-plane owned (512)
    NPAIR = NX // PB

    nc = bacc.Bacc("TRN2", target_bir_lowering=False, debug=False,
                   enable_asserts=False, num_devices=cfg.ncores)

    planes = nc.dram_tensor("planes", [NX * 128, PLF], dt, kind="ExternalInput").ap()
    w1 = nc.dram_tensor("w1", [C, R], dt, kind="ExternalInput").ap()
    w2 = nc.dram_tensor("w2", [R, C], dt, kind="ExternalInput").ap()
    outp = nc.dram_tensor("outp", [NX * 128, VOX2], dt, kind="ExternalOutput").ap()

    with tile.TileContext(nc) as tc, ExitStack() as ctx:
        const = ctx.enter_context(tc.tile_pool(name="const", bufs=1))
        pp = ctx.enter_context(tc.tile_pool(name="pp", bufs=7))
        ztp = ctx.enter_context(tc.tile_pool(name="ztp", bufs=2))
        ytp = ctx.enter_context(tc.tile_pool(name="ytp", bufs=2))
        oyp = ctx.enter_context(tc.tile_pool(name="oyp", bufs=3))
        m2xp = ctx.enter_context(tc.tile_pool(name="m2xp", bufs=4))
        m4xp = ctx.enter_context(tc.tile_pool(name="m4xp", bufs=6))
        pxp = ctx.enter_context(tc.tile_pool(name="pxp", bufs=3))
        hpp = ctx.enter_context(tc.tile_pool(name="hpp", bufs=2, space="PSUM"))
        y2p = ctx.enter_context(tc.tile_pool(name="y2p", bufs=2, space="PSUM"))
        hsp = ctx.enter_context(tc.tile_pool(name="hsp", bufs=3))
        sgp = ctx.enter_context(tc.tile_pool(name="sgp", bufs=5))
        prp = ctx.enter_context(tc.tile_pool(name="prp", bufs=3))

        # ---- constants
        w1sb = const.tile([128, 2 * R], dt)
        nc.sync.dma_start(
            w1sb[:].rearrange("p (h r) -> p h r", h=2),
            w1.rearrange("(h p) r -> p h r", p=128),
        )
        w2sb = const.tile([128, C], dt)
        nc.sync.dma_start(w2sb[:], w2)
        neg = const.tile([128, VOX2], dt)
        nc.gpsimd.memset(neg[:], SENT)
        w1v = w1sb[:].rearrange("p (h r) -> p h r", h=2)

        # rings hold APs (slices of pool tiles); missing entries resolve to neg
        oy_t, m2x_t, m4x_t = {}, {}, {}

        def oy_at(j):
            return oy_t.get(j, neg[:])

        def m2x_at(j):
            return m2x_t.get(j, neg[:])

        def m4x_at(j):
            return m4x_t.get(j, neg[:])

        P_t = {}  # plane tiles for the final multiply (center views)
        Pp_t = {}  # prefetched pair tiles
        sg_t = {}  # sigmoid tiles awaiting the delayed multiply

        def load_pair(pi):
            P = pp.tile([128, PB * PLF], dt)
            for b in range(PB):
                x = pi * PB + b
                nc.sync.dma_start(
                    P[:, b * PLF:(b + 1) * PLF],
                    planes[x * 128:(x + 1) * 128, :],
                )
                P_t[x] = P[:, b * PLF:(b + 1) * PLF]
            Pp_t[pi] = P

        for pi0 in range(2):
            load_pair(pi0)

        for i in range(NX + 6):
            if i < NX and i % PB == 0:
                pi = i // PB
                if pi + 2 < NX // PB:
                    load_pair(pi + 2)
                P = Pp_t.pop(pi)
                # batched view: (pl h) merges into one uniform dim of 2*PB
                Pv = P[:].rearrange("p (g z y) -> p g z y", g=2 * PB, z=ZH)

                # ---- z-pass (window 7 over ZH=14 -> ZS=8), all aligned
                m2z = ztp.tile([128, 2 * PB * (ZH - 1) * YP], dt)
                m2zv = m2z[:].rearrange("p (g z y) -> p g z y", g=2 * PB, z=ZH - 1)
                nc.vector.tensor_max(m2zv, Pv[:, :, 0:ZH - 1, :], Pv[:, :, 1:ZH, :])
                m4z = ztp.tile([128, 2 * PB * (ZH - 3) * YP], dt)
                m4zv = m4z[:].rearrange("p (g z y) -> p g z y", g=2 * PB, z=ZH - 3)
                nc.vector.tensor_max(
                    m4zv, m2zv[:, :, 0:ZH - 3, :], m2zv[:, :, 2:ZH - 1, :]
                )
                z8 = ztp.tile([128, 2 * PB * ZS * YP], dt)
                z8v = z8[:].rearrange("p (g z y) -> p g z y", g=2 * PB, z=ZS)
                nc.vector.tensor_max(
                    z8v, m4zv[:, :, 0:ZS, :], m4zv[:, :, 3:3 + ZS, :]
                )

                # ---- y-pass (window 7 over YP=72 padded -> D=64)
                m2y = ytp.tile([128, 2 * PB * ZS * YP], dt)
                m2yv = m2y[:].rearrange("p (g z y) -> p g z y", g=2 * PB, z=ZS)
                nc.vector.tensor_max(
                    m2yv[:, :, :, 0:YP - 1], z8v[:, :, :, 0:YP - 1],
                    z8v[:, :, :, 1:YP],
                )
                m4y = ytp.tile([128, 2 * PB * ZS * YP], dt)
                m4yv = m4y[:].rearrange("p (g z y) -> p g z y", g=2 * PB, z=ZS)
                nc.vector.tensor_max(
                    m4yv[:, :, :, 0:YP - 3], m2yv[:, :, :, 0:YP - 3],
                    m2yv[:, :, :, 2:YP - 1],
                )
                oy = oyp.tile([128, PB * VOX2], dt)
                oyv = oy[:].rearrange("p (g z y) -> p g z y", g=2 * PB, z=ZS)
                nc.vector.tensor_max(
                    oyv, m4yv[:, :, :, 1:1 + D], m4yv[:, :, :, 4:4 + D]
                )
                for b in range(PB):
                    # per-plane compact [128, VOX2] views: g = (pl, h) so plane
                    # b's halves are g=2b, 2b+1 -> contiguous chunk of VOX2
                    oy_t[i + b] = oy[:, b * VOX2:(b + 1) * VOX2]

            # ---- x-pass (streamed, per plane); negative-index partials give
            # the left-edge clipped windows (right edge clips via aliases)
            j = i - 1
            if j == -1:
                m2x_t[j] = oy_t[0]  # max(oy[-1]=-inf, oy[0])
            elif 0 <= j < NX:
                if j + 1 < NX:
                    m2x = m2xp.tile([128, VOX2], dt)
                    nc.vector.tensor_max(m2x[:], oy_at(j), oy_at(j + 1))
                    m2x_t[j] = m2x[:]
                else:
                    m2x_t[j] = oy_t[j]
            j = i - 3
            if j == -2:
                m4x_t[j] = m2x_t[0]  # max(m2x[-2]=-inf, m2x[0])
            elif j == -1 or (0 <= j < NX - 2):
                m4x = m4xp.tile([128, VOX2], dt)
                nc.vector.tensor_max(m4x[:], m2x_at(j), m2x_at(j + 2))
                m4x_t[j] = m4x[:]
            elif 0 <= j < NX:
                m4x_t[j] = m2x_t[j]
            k = i - 3
            if 0 <= k < NX:
                if k >= 1:
                    px = pxp.tile([128, VOX2], dt)
                    nc.vector.tensor_max(px[:], m4x_at(k - 3), m4x_at(k))
                    pxa = px[:]
                else:
                    pxa = m4x_at(k)

                # ---- MLP on plane k: h = relu(W1^T @ px) on PE+ACT
                pxv = pxa.rearrange("p (h v) -> p h v", h=2)
                hp = hpp.tile([128, VOXH], f32, space="PSUM")
                for h in (0, 1):
                    nc.tensor.matmul(
                        hp[:], w1v[:, h, :], pxv[:, h, :],
                        start=(h == 0), stop=(h == 1),
                    )
                hs = hsp.tile([128, VOXH], dt)
                nc.scalar.activation(hs[:], hp[:], AF.Relu)
                # ---- y2^T = W2^T @ h: output lands [ch-part, vox]
                y2 = y2p.tile([128, 2 * VOXH], f32, space="PSUM")
                for h in (0, 1):
                    nc.tensor.matmul(
                        y2[:, h * VOXH:(h + 1) * VOXH],
                        w2sb[:, h * 128:(h + 1) * 128], hs[:],
                        start=True, stop=True,
                    )
                sg = sgp.tile([128, VOX2], dt)
                nc.scalar.activation(sg[:], y2[:], AF.Sigmoid)
                sg_t[k] = sg

            # ---- dense multiply (delayed 2 steps so the PE/ACT round trip
            # never blocks the in-order DVE queue): prod = sg * plane_center
            k2 = i - 6
            if 0 <= k2 < NX:
                sg = sg_t.pop(k2)
                Pc = P_t.pop(k2).rearrange("p (h z y) -> p h z y", h=2, z=ZH)[
                    :, :, 3:3 + ZS, 4:4 + D
                ]
                prod = prp.tile([128, VOX2], dt)
                prodv = prod[:].rearrange("p (h z y) -> p h z y", h=2, z=ZS)
                nc.vector.tensor_mul(prodv, sg[:].rearrange(
                    "p (h z y) -> p h z y", h=2, z=ZS), Pc)
                nc.gpsimd.dma_start(
                    outp[k2 * 128:(k2 + 1) * 128, :], prod[:]
                )

    nc.compile()
    return nc


def host_prep(cfg: Cfg, feats, coords, W1, W2):
    """Build per-core dense slabs in device layout. Pure data marshalling."""
    D, ZS, ZH, YP, NX = cfg.D, cfg.ZS, cfg.ZH, cfg.YP, cfg.NX
    dt = _np_dt(cfg.dt)

    ix = coords[:, 0].astype(np.int64)
    iy = coords[:, 1].astype(np.int64)
    iz = coords[:, 2].astype(np.int64)

    # dense grid, padded z by 3 each side and y by 4 each side, in layout
    # [x, p(=c%128), h(=c//128), zpad, ypad]
    gridT = np.full((D, 128, 2, D + 6, YP), SENT, dtype=dt)
    fsplit = feats.astype(dt).reshape(-1, 2, 128)  # [N, h, p]
    gridT[ix, :, :, iz + 3, iy + 4] = fsplit.transpose(0, 2, 1)

    w1h = np.ascontiguousarray(W1.astype(dt))
    w2h = np.ascontiguousarray(W2.astype(dt))

    in_maps = []
    for k in range(cfg.ncores):
        slab = np.ascontiguousarray(gridT[:, :, :, 8 * k:8 * k + ZH, :])
        in_maps.append({
            "planes": slab.reshape(NX * 128, cfg.PLF),
            "w1": w1h,
            "w2": w2h,
        })
    return in_maps


def host_post(cfg: Cfg, results, coords):
    """Gather occupied voxels from the dense product planes."""
    D, ZS, NX = cfg.D, cfg.ZS, cfg.NX
    ix = coords[:, 0].astype(np.int64)
    iy = coords[:, 1].astype(np.int64)
    iz = coords[:, 2].astype(np.int64)
    out = np.empty((cfg.NPTS, C), np.float32)
    for k in range(cfg.ncores):
        sel = np.where((iz >= k * ZS) & (iz < (k + 1) * ZS))[0]
        pk = np.asarray(results[k]["outp"]).reshape(NX, 128, 2, ZS, D)
        # value for point n at channel c = h*128+p: pk[ix, p, h, iz%8, iy]
        v = pk[ix[sel], :, :, iz[sel] - k * ZS, iy[sel]]  # [n, 128, 2]
        out[sel] = v.transpose(0, 2, 1).reshape(len(sel), C).astype(np.float32)
    return out


_CACHE = {}


def _get_nc(cfg: Cfg):
    if cfg not in _CACHE:
        _CACHE[cfg] = build_nc(cfg)
    return _CACHE[cfg]


def kernel(feats, coords, W1, W2):
    from concourse.bass_utils import run_bass_kernel_spmd

    cfg = FULL
    nc = _get_nc(cfg)
    in_maps = host_prep(
        cfg,
        np.asarray(feats, np.float32),
        np.asarray(coords),
        np.asarray(W1, np.float32),
        np.asarray(W2, np.float32),
    )
    res = run_bass_kernel_spmd(nc, in_maps, core_ids=list(range(cfg.ncores)))
    return host_post(cfg, res.results, np.asarray(coords))
